# revision 30
# baseline (speedup 1.0000x reference)
"""Distributional twin-critic MLP forward, data-parallel over 8 NeuronCores.

Math (per critic c, eval mode):
    x   = concat(state, action)                       [B, 576]
    h   = relu(LN(x @ W_f1.T + b_f1) * g1 + beta1)    [B, 1024]
    f   = relu(LN(h @ W_f2.T + b_f2) * g2 + beta2)    [B, 1024]
    q   = f @ wh_feat + te @ wh_tau + b_h             [B, NQ] (outer sum)

Device strategy (pure data parallel, batch shard 2048 rows per core), on
top of the previous fp32r version:
  - all matmuls in bf16 (fp32 PSUM accumulation).  fp32r's 4-byte
    LDWEIGHTS exposed ~24 ns per matmul on the weight-load port; bf16
    loads are fully hidden under the 213 ns N=512 stream.  bf16 also
    halves the HBM prologue (9 MB total vs 18) and all SBUF traffic.
    Host-simulated end-to-end max-err/scale ~6e-3 (gate 2e-2).
  - LN mean folded into centered weights (host); on-chip LN is an
    RMS-norm via all-constant stationary matmuls (partition broadcast).
  - layer-1 rstd needs only a cheap estimate: layer-2's RMS-norm is
    scale-invariant per sample, so rstd1 errors cancel except through
    the tiny b2 coupling.  Variance is taken over feature m-tiles {2,5}
    (256 of 1024 features) -> 2 stats matmuls per critic instead of 8.
  - the two critics interleave per m-tile, which lets their K=64 L1
    tail matmuls (state 512 = 4 full K-tiles, action 64 = tail; bias
    moved into the PSUM evictions) row-pack via tile_position (0,*) /
    (64,*) and run concurrently -- 2 tails in one matmul slot.
  - layer-2 stats and the M=64 head col-pack: critic0 accumulates on
    PSUM partitions 0-63, critic1 on 64-127, heads in one bank, stats
    in another; adjacent col-disjoint pairs (head_c0 | stats_c1) run
    concurrently -- 4 matmuls in 2 slots.  rstd2 then lands on the head
    output on matching partitions (engines are lane-locked).
  - software pipelined skew: emit L1(bt) then L2(bt-1), so each tile's
    stats->rsqrt->scale chain (ACT/DVE) executes under the previous
    tile's layer-2 matmul stream instead of stalling the PE.
  - evictions are fused: z2 = Square(psum + b) on ACT, relu via a
    single DVE tensor_scalar (add bias, max 0) -> bf16.
  - tau embedding is batch-independent -> host (64x64 chain).

Matmul budget per core: 4 tiles x (64 L1 + 8 packed tails + 4 L1 stats
+ 128 L2 + 16 packed head/stats) = 880 slots x ~216 ns ~= 190 us.
"""

import os
import sys

import numpy as np

sys.path.insert(0, "/opt/trn_rl_repo")

import concourse.bacc as bacc
import concourse.tile as tile
from concourse import mybir
from concourse.bass_utils import run_bass_kernel_spmd

try:
    import ml_dtypes

    BF16_NP = ml_dtypes.bfloat16
except ImportError:  # pragma: no cover
    BF16_NP = None

F32 = mybir.dt.float32
BF = mybir.dt.bfloat16
AF = mybir.ActivationFunctionType
AL = mybir.AluOpType

B, SD, AD, H, QE, NQ = 16384, 512, 64, 1024, 64, 64
D = SD + AD                      # 576
NCORES = 8
BSH = B // NCORES                # 2048 batch rows per core
NT = 512                         # batch tile (matmul free dim)
NBT = BSH // NT                  # 4
NM = H // 128                    # 8 M-tiles (and K-tiles for layer 2)
STATS_M = (2, 5)                 # L1 variance sample m-tiles (256 feats)
EPS = 1e-5
NWARM = 8                        # HAM warmup matmuls (~3.4us cold)

_CACHE = {}
_LAST_RESULT = None


def _build(unit_affine):
    nc = bacc.Bacc("TRN2", target_bir_lowering=False, debug=False,
                   num_devices=NCORES)

    xT = nc.dram_tensor("xT", [D, BSH], BF, kind="ExternalInput").ap()
    w1 = nc.dram_tensor("w1", [2, SD, H], BF, kind="ExternalInput").ap()
    w1t = nc.dram_tensor("w1t", [128, H], BF, kind="ExternalInput").ap()
    w2 = nc.dram_tensor("w2", [2, H, H], BF, kind="ExternalInput").ap()
    whr = nc.dram_tensor("whr", [2, H, 64], BF, kind="ExternalInput").ap()
    # per-feature vectors arranged [c, p, vec, m] with feature = m*128 + p
    vecs = nc.dram_tensor("vecs", [2, 128, 6, NM], F32,
                          kind="ExternalInput").ap()
    qtb = nc.dram_tensor("qtb", [2, 64, 1], F32, kind="ExternalInput").ap()
    out_q = nc.dram_tensor("out_q", [2, NQ, BSH], F32,
                           kind="ExternalOutput").ap()

    with tile.TileContext(nc) as tc:
        with tc.tile_pool(name="wpool", bufs=1) as wp, \
             tc.tile_pool(name="xpool", bufs=2) as xp, \
             tc.tile_pool(name="zpool", bufs=1) as zp_, \
             tc.tile_pool(name="hpool", bufs=1) as hp, \
             tc.tile_pool(name="spool", bufs=1) as sp_, \
             tc.tile_pool(name="zpsum", bufs=5, space="PSUM") as zps, \
             tc.tile_pool(name="apsum", bufs=3, space="PSUM") as aux:

            # ---- resident weights, consumption order, 2 queues ----
            w1b = [wp.tile([128, 4, H], BF, tag=f"w1b_{c}", name=f"w1b_{c}")
                   for c in range(2)]
            w1tt = wp.tile([128, H], BF, tag="w1t", name="w1tt")
            w2b = [[wp.tile([128, 4, H], BF, tag=f"w2b_{c}_{j}",
                            name=f"w2b_{c}_{j}") for j in range(2)]
                   for c in range(2)]
            wht = [wp.tile([128, NM, 64], BF, tag=f"wh_{c}", name=f"wh_{c}")
                   for c in range(2)]
            vt = [wp.tile([128, 6, NM], F32, tag=f"vec_{c}", name=f"vec_{c}")
                  for c in range(2)]
            qtbv = wp.tile([128, 1], F32, tag="qtbv", name="qtbv")
            qtbg = wp.tile([128, 2], F32, tag="qtbg", name="qtbg")

            # ---- on-chip constants + HAM warmup first (no DMA deps, and
            # the DVE queue must run the memsets before its DMA triggers) ----
            mtL = wp.tile([128, 128], BF, tag="mtL", name="mtL")
            nc.vector.memset(mtL[:], 1.0 / (128 * len(STATS_M)))
            mt64 = wp.tile([128, 64], BF, tag="mt64", name="mt64")
            nc.vector.memset(mt64[:], 1.0 / H)
            mtG = wp.tile([128, 128], BF, tag="mtG", name="mtG")
            nc.vector.memset(mtG[:], 1.0 / H)
            epst = wp.tile([128, 1], F32, tag="epst", name="epst")
            nc.vector.memset(epst[:], EPS)
            wmv = wp.tile([128, NT], BF, tag="wmv", name="wmv")
            nc.vector.memset(wmv[:], 0.0)
            wq = zps.tile([128, NT], F32, tag="zp", name="wq")
            for _ in range(NWARM):
                nc.tensor.matmul(wq[:], mtL[:], wmv[:], start=True, stop=True)

            # ---- input / weight DMAs ----
            # x comes in 2-batch-tile chunks so the per-partition DMA line
            # stays 2KB (bf16 at NT=512 would be 1KB lines = half-rate DMA)
            def x_dma(chunk):
                b0 = chunk * 2 * NT
                xk = []
                for k in range(4):
                    t = xp.tile([128, 2 * NT], BF, tag=f"x{k}", name=f"x{k}")
                    nc.sync.dma_start(out=t[:],
                                      in_=xT[k * 128:(k + 1) * 128,
                                             b0:b0 + 2 * NT])
                    xk.append(t)
                xtl = xp.tile([128, 2 * NT], BF, tag="xt", name="xtl")
                nc.sync.dma_start(out=xtl[0:64, :],
                                  in_=xT[SD:D, b0:b0 + 2 * NT])
                nc.sync.dma_start(out=xtl[64:128, :],
                                  in_=xT[SD:D, b0:b0 + 2 * NT])
                return xk, xtl

            # 3 queues, consumption order.  Critical path for the first
            # m-blocks is x chunk0 (sync) + w1c0 split across gpsimd/scalar;
            # critic 1's w1 lands under critic 0's compute (bt=0 runs the
            # critics sequentially for exactly this reason).
            xchunk = {0: x_dma(0)}
            # per-k w1 DMAs (2KB lines) so the m=0 K-chain starts as soon
            # as its first k-tile lands, alternating gpsimd/scalar
            for k in range(4):
                eng = nc.gpsimd if k % 2 == 0 else nc.scalar
                eng.dma_start(out=w1b[0][:, k, :],
                              in_=w1[0, k * 128:(k + 1) * 128, :])
            nc.gpsimd.dma_start(out=w1tt[:], in_=w1t)
            for k in range(4):
                eng = nc.sync if k % 2 == 0 else nc.scalar
                eng.dma_start(out=w1b[1][:, k, :],
                              in_=w1[1, k * 128:(k + 1) * 128, :])
            for c in range(2):
                eng = nc.gpsimd if c == 0 else nc.scalar
                for j in range(2):
                    eng.dma_start(
                        out=w2b[c][j][:],
                        in_=w2[c, j * 512:(j + 1) * 512, :]
                        .rearrange("(a p) h -> p a h", p=128))
                eng.dma_start(
                    out=wht[c][:],
                    in_=whr[c].rearrange("(a p) h -> p a h", p=128))
                eng.dma_start(out=vt[c][:], in_=vecs[c])
                eng.dma_start(out=qtbv[c * 64:(c + 1) * 64, :], in_=qtb[c])
                eng.dma_start(out=qtbg[0:64, c:c + 1], in_=qtb[c])

            def x_for(bt):
                if bt % 2 == 0 and bt // 2 not in xchunk:
                    xchunk[bt // 2] = x_dma(bt // 2)
                xk, xtl = xchunk[bt // 2]
                off = (bt % 2) * NT
                return [t[:, off:off + NT] for t in xk], xtl, off

            def w1_ap(c, k, m):
                return w1b[c][:, k, m * 128:(m + 1) * 128]

            def w2_ap(c, k, m):
                return w2b[c][k // 4][:, k % 4, m * 128:(m + 1) * 128]

            def b_ap(c, i, m):
                # vt layout [p, vec_idx, m]; vec order: b1,g1,be1,b2,g2,be2
                return vt[c][:, i, m:m + 1]

            def rsqrt(dst, src, bias):
                nc.scalar.activation(dst, src, AF.Abs_reciprocal_sqrt,
                                     bias=bias)

            def relu_evict(c, z, zpm, bias):
                # split across engines: DVE takes critic 0, ACT critic 1,
                # so the two evictions of an m-block run in parallel
                if c == 0:
                    nc.vector.tensor_scalar(z[:], zpm[:], bias, 0.0,
                                            AL.add, AL.max)
                else:
                    nc.scalar.activation(z[:], zpm[:], AF.Relu, bias=bias)

            # rstd1 scales go through a deferred queue drained 2-per-m-block
            # inside the NEXT emission section, so they interleave with that
            # section's PSUM evictions in the DVE FIFO instead of blocking
            # them (head-of-line starvation of the zp ring)
            mul_q = []

            def drain_muls(n):
                for _ in range(min(n, len(mul_q))):
                    z, rs = mul_q.pop(0)
                    nc.vector.tensor_mul(z[:], z[:], rs[:])

            # ---------------- fast path (g == 1, beta == 0) ----------------
            def emit_l1(bt):
                """Layer 1 for both critics; returns the scaled bf16 h tiles
                (mul by rstd1 emitted here, executes under the previous
                tile's L2 stream).  bt=0 runs the critics sequentially with
                the K=64 tails deferred 2 m-blocks (DMA-paced prologue);
                later tiles interleave per m and row-pack the tails."""
                xk, xtlt, off = x_for(bt)
                zs = {0: [None] * NM, 1: [None] * NM}
                sp = {}
                pend = []

                def tail_ap(c, m):
                    return (w1tt[64 * c:64 * c + 64, m * 128:(m + 1) * 128],
                            xtlt[64 * c:64 * c + 64, off:off + NT])

                def evict(c, m, zpm):
                    z = hp.tile([128, NT], BF, tag=f"z{c}{m}",
                                name=f"z{c}{m}", bufs=2)
                    relu_evict(c, z, zpm, b_ap(c, 0, m))
                    if m in STATS_M:
                        z2 = zp_.tile([128, NT], BF, tag=f"z2{c}",
                                      name=f"z2{c}", bufs=2)
                        nc.scalar.activation(z2[:], zpm[:], AF.Square,
                                             bias=b_ap(c, 0, m))
                        pend.append((c, m, z2))
                    zs[c][m] = z

                def flush(upto, c=None):
                    # pair-sum the two sampled z2 m-tiles on gpsimd, then a
                    # single all-(1/256) stats matmul per critic
                    byc = {}
                    for pc, ms, z2 in pend:
                        byc.setdefault(pc, []).append((ms, z2))
                    for pc, entries in byc.items():
                        if c is not None and pc != c:
                            continue
                        if len(entries) < len(STATS_M) or \
                                entries[-1][0] > upto:
                            continue
                        zsum = zp_.tile([128, NT], BF, tag=f"zs{pc}",
                                        name=f"zs{pc}", bufs=2)
                        nc.gpsimd.tensor_add(zsum[:], entries[0][1][:],
                                             entries[1][1][:])
                        sp[pc] = aux.tile([128, NT], F32, tag="aux",
                                          name=f"sp{pc}")
                        nc.tensor.matmul(sp[pc][:], mtL[:], zsum[:],
                                         start=True, stop=True)
                        pend[:] = [e for e in pend if e[0] != pc]

                def scale(c):
                    rs = sp_.tile([128, NT], BF, tag=f"rs{c}", name=f"rs{c}",
                                  bufs=2)
                    rsqrt(rs[:], sp[c][:], epst[:])
                    for m in range(NM):
                        mul_q.append((zs[c][m], rs))

                if bt == 0:
                    for c in range(2):
                        opened = []

                        def finish(m, zpm, c=c):
                            wt, xt = tail_ap(c, m)
                            nc.tensor.matmul(zpm[:], wt, xt, start=False,
                                             stop=True)
                            evict(c, m, zpm)

                        for m in range(NM):
                            drain_muls(2)
                            zpm = zps.tile([128, NT], F32, tag="zp",
                                           name="zp")
                            for k in range(4):
                                nc.tensor.matmul(zpm[:], w1_ap(c, k, m),
                                                 xk[k], start=(k == 0),
                                                 stop=False)
                            opened.append((m, zpm))
                            if len(opened) > 2:
                                finish(*opened.pop(0))
                        while opened:
                            finish(*opened.pop(0))
                        flush(NM, c)
                        scale(c)
                    return zs

                for m in range(NM):
                    drain_muls(2)
                    zpm = {}
                    for c in range(2):
                        zpm[c] = zps.tile([128, NT], F32, tag="zp",
                                          name="zp")
                        for k in range(4):
                            nc.tensor.matmul(zpm[c][:], w1_ap(c, k, m),
                                             xk[k], start=(k == 0),
                                             stop=False)
                    # K=64 action tails, row-packed across the critics
                    for c in range(2):
                        wt, xt = tail_ap(c, m)
                        nc.tensor.matmul(zpm[c][:], wt, xt, start=False,
                                         stop=True)
                    for c in range(2):
                        evict(c, m, zpm[c])
                    flush(m - 2)
                flush(NM)
                for c in range(2):
                    scale(c)
                return zs

            def emit_l2(bt, zs):
                b0 = bt * NT
                S = aux.tile([128, NT], F32, tag="aux", name="S")
                Hb = aux.tile([128, NT], F32, tag="aux", name="Hb")
                ffs = {0: [None] * NM, 1: [None] * NM}
                z2f = {0: [None] * NM, 1: [None] * NM}

                zfp = {0: [None] * (NM // 2), 1: [None] * (NM // 2)}

                def flush(upto):
                    # heads of the two critics are col-disjoint (partitions
                    # 0-63 vs 64-127) and run concurrently; stats likewise,
                    # on pair-summed z2f tiles (4 matmuls per critic)
                    for j in range(flush.done + 1, min(upto, NM - 1) + 1):
                        nc.tensor.matmul(Hb[0:64, :], wht[0][:, j, :],
                                         ffs[0][j][:], start=(j == 0),
                                         stop=(j == NM - 1))
                        nc.tensor.matmul(Hb[64:128, :], wht[1][:, j, :],
                                         ffs[1][j][:], start=(j == 0),
                                         stop=(j == NM - 1))
                        if j % 2 == 1:
                            jj = j // 2
                            nc.tensor.matmul(S[0:64, :], mt64[:],
                                             zfp[0][jj][:], start=(jj == 0),
                                             stop=(jj == NM // 2 - 1))
                            nc.tensor.matmul(S[64:128, :], mt64[:],
                                             zfp[1][jj][:], start=(jj == 0),
                                             stop=(jj == NM // 2 - 1))
                        flush.done = j
                flush.done = -1

                for m in range(NM):
                    drain_muls(2)
                    zpm = {}
                    for c in range(2):
                        zpm[c] = zps.tile([128, NT], F32, tag="zp",
                                          name="zp2")
                        for k in range(NM):
                            nc.tensor.matmul(zpm[c][:], w2_ap(c, k, m),
                                             zs[c][k][:], start=(k == 0),
                                             stop=(k == NM - 1))
                    for c in range(2):
                        ff = hp.tile([128, NT], BF, tag=f"f{c}{m}",
                                     name=f"f{c}{m}", bufs=1)
                        relu_evict(c, ff, zpm[c], b_ap(c, 3, m))
                        zq = zp_.tile([128, NT], BF, tag=f"zf{c}",
                                      name=f"zf{c}", bufs=3)
                        nc.scalar.activation(zq[:], zpm[c][:], AF.Square,
                                             bias=b_ap(c, 3, m))
                        ffs[c][m] = ff
                        z2f[c][m] = zq
                        if m % 2 == 1:
                            zp2t = zp_.tile([128, NT], BF, tag=f"zfp{c}",
                                            name=f"zfp{c}", bufs=2)
                            nc.gpsimd.tensor_add(zp2t[:], z2f[c][m - 1][:],
                                                 zq[:])
                            zfp[c][m // 2] = zp2t
                    flush(m - 2)
                flush(NM)
                # rstd2 lands on the head output (RMS scale invariance);
                # critic c lives on partitions [64c, 64c+64).  Stage-outer
                # + column-halved so the exposed serial tail (last tile)
                # pipelines: rsqrt (ACT) -> mul/bias (DVE) -> DMA per half.
                rs2 = sp_.tile([128, NT], F32, tag="rs2", name="rs2")
                q0 = sp_.tile([128, NT], F32, tag="q0", name="q0")
                qf = sp_.tile([128, NT], F32, tag="qf", name="qf")
                for n0 in (0, NT // 2):
                    cols = slice(n0, n0 + NT // 2)
                    for c in range(2):
                        lo, hi = 64 * c, 64 * c + 64
                        rsqrt(rs2[lo:hi, cols], S[lo:hi, cols],
                              epst[lo:hi, :])
                    for c in range(2):
                        lo, hi = 64 * c, 64 * c + 64
                        nc.vector.tensor_mul(q0[lo:hi, cols], Hb[lo:hi, cols],
                                             rs2[lo:hi, cols])
                        nc.vector.tensor_scalar_add(qf[lo:hi, cols],
                                                    q0[lo:hi, cols],
                                                    qtbv[lo:hi, :])
                        nc.sync.dma_start(
                            out=out_q[c, :, b0 + n0:b0 + n0 + NT // 2],
                            in_=qf[lo:hi, cols])

            def emit_fast():
                carry = None
                for bt in range(NBT + 1):
                    zs = emit_l1(bt) if bt < NBT else None
                    if carry is not None:
                        emit_l2(bt - 1, carry)
                    carry = zs

            # ------------- general path (arbitrary g / beta) -------------
            def gen_block(c, act, wts_of_m, nk, layer, tail=None):
                zs = []
                sp = aux.tile([128, NT], F32, tag="aux", name="sp")
                pend = []

                def flush(upto):
                    while pend and pend[0][0] <= upto:
                        m, z2 = pend.pop(0)
                        nc.tensor.matmul(sp[:], mtG[:], z2[:],
                                         start=(m == 0), stop=(m == NM - 1))

                b_i = 0 if layer == 0 else 3
                for m in range(NM):
                    zpm = zps.tile([128, NT], F32, tag="zp", name="zpg")
                    for k in range(nk):
                        nc.tensor.matmul(zpm[:], wts_of_m(k, m), act[k],
                                         start=(k == 0),
                                         stop=(k == nk - 1 and tail is None))
                    if tail is not None:
                        wt, xt = tail
                        nc.tensor.matmul(zpm[:],
                                         wt[:, m * 128:(m + 1) * 128],
                                         xt, start=False, stop=True)
                    z = zp_.tile([128, NT], F32, tag=f"zg{m}", name=f"zg{m}")
                    nc.vector.tensor_scalar_add(z[:], zpm[:], b_ap(c, b_i, m))
                    z2 = zp_.tile([128, NT], BF, tag=f"z2g_{m % 3}",
                                  name=f"z2g{m % 3}", bufs=1)
                    nc.scalar.activation(z2[:], zpm[:], AF.Square,
                                         bias=b_ap(c, b_i, m))
                    pend.append((m, z2))
                    flush(m - 2)
                    zs.append(z)
                flush(NM)
                return zs, sp

            def gen_norm(c, zs, sp, layer):
                g_i, be_i = (1, 2) if layer == 0 else (4, 5)
                rs = sp_.tile([128, NT], F32, tag="rsg", name="rsg")
                rsqrt(rs[:], sp[:], epst[:])
                hs = []
                for m in range(NM):
                    nc.vector.tensor_mul(zs[m][:], zs[m][:], rs[:])
                    ht = hp.tile([128, NT], BF, tag=f"hg{m}", name=f"hg{m}")
                    nc.scalar.activation(ht[:], zs[m][:], AF.Relu,
                                         bias=b_ap(c, be_i, m),
                                         scale=b_ap(c, g_i, m))
                    hs.append(ht)
                return hs

            def emit_general():
                for bt in range(NBT):
                    b0 = bt * NT
                    xk, xtlt, off = x_for(bt)
                    for c in range(2):
                        tail = (w1tt[64 * c:64 * c + 64, :],
                                xtlt[64 * c:64 * c + 64, off:off + NT])
                        zs, sp = gen_block(
                            c, xk, lambda k, m, c=c: w1_ap(c, k, m), 4, 0,
                            tail=tail)
                        h1 = gen_norm(c, zs, sp, 0)
                        zs, sp = gen_block(
                            c, [t[:] for t in h1],
                            lambda k, m, c=c: w2_ap(c, k, m), NM, 1)
                        ff = gen_norm(c, zs, sp, 1)
                        qp = aux.tile([128, NT], F32, tag="aux", name="qp")
                        for k in range(NM):
                            nc.tensor.matmul(qp[0:64, :], wht[c][:, k, :],
                                             ff[k][:], start=(k == 0),
                                             stop=(k == NM - 1))
                        qf = sp_.tile([128, NT], F32, tag="qfg", name="qfg",
                                      bufs=2)
                        nc.scalar.activation(qf[0:64, :], qp[0:64, :],
                                             AF.Identity,
                                             bias=qtbg[0:64, c:c + 1])
                        nc.gpsimd.dma_start(out=out_q[c, :, b0:b0 + NT],
                                            in_=qf[0:64, :])

            if unit_affine:
                emit_fast()
            else:
                emit_general()
    nc.compile()
    return nc


def _prep_host(inputs):
    state = np.ascontiguousarray(inputs["state"], dtype=np.float32)
    action = np.ascontiguousarray(inputs["action"], dtype=np.float32)
    W_f1 = np.asarray(inputs["W_f1"], np.float32)
    b_f1 = np.asarray(inputs["b_f1"], np.float32)
    g1 = np.asarray(inputs["g1"], np.float32)
    beta1 = np.asarray(inputs["beta1"], np.float32)
    W_f2 = np.asarray(inputs["W_f2"], np.float32)
    b_f2 = np.asarray(inputs["b_f2"], np.float32)
    g2 = np.asarray(inputs["g2"], np.float32)
    beta2 = np.asarray(inputs["beta2"], np.float32)
    W_h = np.asarray(inputs["W_h"], np.float32)
    b_h = np.asarray(inputs["b_h"], np.float32)
    W_e1 = np.asarray(inputs["W_e1"], np.float32)
    b_e1 = np.asarray(inputs["b_e1"], np.float32)
    W_e2 = np.asarray(inputs["W_e2"], np.float32)
    b_e2 = np.asarray(inputs["b_e2"], np.float32)

    unit_affine = (np.all(g1 == 1.0) and np.all(beta1 == 0.0)
                   and np.all(g2 == 1.0) and np.all(beta2 == 0.0))

    x = np.concatenate([state, action], axis=1)          # [B, 576]
    xT = np.ascontiguousarray(x.T).astype(BF16_NP)       # [576, B] bf16

    # transpose weights and fold the LN mean subtraction into them:
    # centering the columns of W.T (and the bias) makes mean_h(z) == 0.
    w1tr = np.ascontiguousarray(W_f1.transpose(0, 2, 1))  # [2, D, H]
    w1c = w1tr - w1tr.mean(axis=2, keepdims=True)
    b1c = b_f1 - b_f1.mean(axis=1, keepdims=True)         # [2, H]
    w2tr = np.ascontiguousarray(W_f2.transpose(0, 2, 1))  # [2, H, H]
    w2c = w2tr - w2tr.mean(axis=2, keepdims=True)
    b2c = b_f2 - b_f2.mean(axis=1, keepdims=True)         # [2, H]

    w1main = np.ascontiguousarray(w1c[:, :SD, :]).astype(BF16_NP)
    w1tail = np.ascontiguousarray(
        np.concatenate([w1c[0, SD:D, :], w1c[1, SD:D, :]], axis=0)
    ).astype(BF16_NP)                                     # [128, H]
    w2b = np.ascontiguousarray(w2c).astype(BF16_NP)

    def as_pm(v):                                        # [2, H] -> [2,128,NM]
        return v.reshape(2, NM, 128).transpose(0, 2, 1)

    vecs = np.ascontiguousarray(np.stack(
        [as_pm(b1c), as_pm(g1), as_pm(beta1),
         as_pm(b2c), as_pm(g2), as_pm(beta2)],
        axis=1).transpose(0, 2, 1, 3))                   # [2, 128, 6, NM]

    wh_feat = W_h[:, 0, :H]                              # [2, H]
    whr = np.ascontiguousarray(
        np.broadcast_to(wh_feat[:, :, None], (2, H, 64)).copy()
    ).astype(BF16_NP)

    # tau embedding: batch-independent, tiny -> host
    tau = (np.linspace(0.0, 1.0, NQ + 1, dtype=np.float32)[:-1]
           + np.float32(1.0 / (2 * NQ)))[:, None]        # [NQ, 1]
    qtb = np.empty((2, 64, 1), np.float32)
    for c in range(2):
        te = np.maximum(tau @ W_e1[c].T + b_e1[c], 0.0) @ W_e2[c].T + b_e2[c]
        qtb[c, :, 0] = te @ W_h[c, 0, H:] + b_h[c, 0]

    shared = {"w1": w1main, "w1t": w1tail, "w2": w2b, "whr": whr,
              "vecs": np.ascontiguousarray(vecs), "qtb": qtb}
    return xT, shared, unit_affine


def kernel(**inputs):
    global _LAST_RESULT
    xT, shared, unit_affine = _prep_host(inputs)
    key = ("nc", unit_affine)
    if key not in _CACHE:
        _CACHE[key] = _build(unit_affine)
    nc = _CACHE[key]

    in_maps = []
    for c in range(NCORES):
        m = dict(shared)
        m["xT"] = np.ascontiguousarray(xT[:, c * BSH:(c + 1) * BSH])
        in_maps.append(m)

    trace = bool(os.environ.get("KERNEL_TRACE"))
    res = run_bass_kernel_spmd(nc, in_maps, list(range(NCORES)), trace=trace)
    _LAST_RESULT = res

    q = np.concatenate([res.results[i]["out_q"] for i in range(NCORES)],
                       axis=2)                           # [2, NQ, B]
    q = np.ascontiguousarray(q.transpose(0, 2, 1))       # [2, B, NQ]
    return q[0], q[1]


# revision 31
# speedup vs baseline: 1.0160x; 1.0160x over previous
"""Distributional twin-critic MLP forward, data-parallel over 8 NeuronCores.

Math (per critic c, eval mode):
    x   = concat(state, action)                       [B, 576]
    h   = relu(LN(x @ W_f1.T + b_f1) * g1 + beta1)    [B, 1024]
    f   = relu(LN(h @ W_f2.T + b_f2) * g2 + beta2)    [B, 1024]
    q   = f @ wh_feat + te @ wh_tau + b_h             [B, NQ] (outer sum)

Device strategy (pure data parallel, batch shard 2048 rows per core), on
top of the previous fp32r version:
  - all matmuls in bf16 (fp32 PSUM accumulation).  fp32r's 4-byte
    LDWEIGHTS exposed ~24 ns per matmul on the weight-load port; bf16
    loads are fully hidden under the 213 ns N=512 stream.  bf16 also
    halves the HBM prologue (9 MB total vs 18) and all SBUF traffic.
    Host-simulated end-to-end max-err/scale ~6e-3 (gate 2e-2).
  - LN mean folded into centered weights (host); on-chip LN is an
    RMS-norm via all-constant stationary matmuls (partition broadcast).
  - layer-1 rstd needs only a cheap estimate: layer-2's RMS-norm is
    scale-invariant per sample, so rstd1 errors cancel except through
    the tiny b2 coupling.  Variance is taken over feature m-tiles {2,5}
    (256 of 1024 features) -> 2 stats matmuls per critic instead of 8.
  - the two critics interleave per m-tile, which lets their K=64 L1
    tail matmuls (state 512 = 4 full K-tiles, action 64 = tail; bias
    moved into the PSUM evictions) row-pack via tile_position (0,*) /
    (64,*) and run concurrently -- 2 tails in one matmul slot.
  - layer-2 stats and the M=64 head col-pack: critic0 accumulates on
    PSUM partitions 0-63, critic1 on 64-127, heads in one bank, stats
    in another; adjacent col-disjoint pairs (head_c0 | stats_c1) run
    concurrently -- 4 matmuls in 2 slots.  rstd2 then lands on the head
    output on matching partitions (engines are lane-locked).
  - software pipelined skew: emit L1(bt) then L2(bt-1), so each tile's
    stats->rsqrt->scale chain (ACT/DVE) executes under the previous
    tile's layer-2 matmul stream instead of stalling the PE.
  - evictions are fused: z2 = Square(psum + b) on ACT, relu via a
    single DVE tensor_scalar (add bias, max 0) -> bf16.
  - tau embedding is batch-independent -> host (64x64 chain).

Matmul budget per core: 4 tiles x (64 L1 + 8 packed tails + 4 L1 stats
+ 128 L2 + 16 packed head/stats) = 880 slots x ~216 ns ~= 190 us.
"""

import os
import sys

import numpy as np

sys.path.insert(0, "/opt/trn_rl_repo")

import concourse.bacc as bacc
import concourse.tile as tile
from concourse import mybir
from concourse.bass_utils import run_bass_kernel_spmd

try:
    import ml_dtypes

    BF16_NP = ml_dtypes.bfloat16
except ImportError:  # pragma: no cover
    BF16_NP = None

F32 = mybir.dt.float32
BF = mybir.dt.bfloat16
AF = mybir.ActivationFunctionType
AL = mybir.AluOpType

B, SD, AD, H, QE, NQ = 16384, 512, 64, 1024, 64, 64
D = SD + AD                      # 576
NCORES = 8
BSH = B // NCORES                # 2048 batch rows per core
NT = 512                         # batch tile (matmul free dim)
NBT = BSH // NT                  # 4
NM = H // 128                    # 8 M-tiles (and K-tiles for layer 2)
STATS_M = (2, 5)                 # L1 variance sample m-tiles (256 feats)
EPS = 1e-5
NWARM = 8                        # HAM warmup matmuls (~3.4us cold)

_CACHE = {}
_LAST_RESULT = None


def _build(unit_affine):
    nc = bacc.Bacc("TRN2", target_bir_lowering=False, debug=False,
                   num_devices=NCORES)

    xT = nc.dram_tensor("xT", [D, BSH], BF, kind="ExternalInput").ap()
    w1 = nc.dram_tensor("w1", [2, SD, H], BF, kind="ExternalInput").ap()
    w1t = nc.dram_tensor("w1t", [128, H], BF, kind="ExternalInput").ap()
    w2 = nc.dram_tensor("w2", [2, H, H], BF, kind="ExternalInput").ap()
    whr = nc.dram_tensor("whr", [2, H, 64], BF, kind="ExternalInput").ap()
    # per-feature vectors arranged [c, p, vec, m] with feature = m*128 + p
    vecs = nc.dram_tensor("vecs", [2, 128, 6, NM], F32,
                          kind="ExternalInput").ap()
    qtb = nc.dram_tensor("qtb", [2, 64, 1], F32, kind="ExternalInput").ap()
    out_q = nc.dram_tensor("out_q", [2, NQ, BSH], F32,
                           kind="ExternalOutput").ap()

    with tile.TileContext(nc) as tc:
        with tc.tile_pool(name="wpool", bufs=1) as wp, \
             tc.tile_pool(name="xpool", bufs=2) as xp, \
             tc.tile_pool(name="zpool", bufs=1) as zp_, \
             tc.tile_pool(name="hpool", bufs=1) as hp, \
             tc.tile_pool(name="spool", bufs=1) as sp_, \
             tc.tile_pool(name="zpsum", bufs=5, space="PSUM") as zps, \
             tc.tile_pool(name="apsum", bufs=3, space="PSUM") as aux:

            # ---- resident weights, consumption order, 2 queues ----
            w1b = [wp.tile([128, 4, H], BF, tag=f"w1b_{c}", name=f"w1b_{c}")
                   for c in range(2)]
            w1tt = wp.tile([128, H], BF, tag="w1t", name="w1tt")
            w2b = [[wp.tile([128, 4, H], BF, tag=f"w2b_{c}_{j}",
                            name=f"w2b_{c}_{j}") for j in range(2)]
                   for c in range(2)]
            wht = [wp.tile([128, NM, 64], BF, tag=f"wh_{c}", name=f"wh_{c}")
                   for c in range(2)]
            vt = [wp.tile([128, 6, NM], F32, tag=f"vec_{c}", name=f"vec_{c}")
                  for c in range(2)]
            qtbv = wp.tile([128, 1], F32, tag="qtbv", name="qtbv")
            qtbg = wp.tile([128, 2], F32, tag="qtbg", name="qtbg")

            # ---- on-chip constants + HAM warmup first (no DMA deps, and
            # the DVE queue must run the memsets before its DMA triggers) ----
            mtL = wp.tile([128, 128], BF, tag="mtL", name="mtL")
            nc.vector.memset(mtL[:], 1.0 / (128 * len(STATS_M)))
            mt64 = wp.tile([128, 64], BF, tag="mt64", name="mt64")
            nc.vector.memset(mt64[:], 1.0 / H)
            mtG = wp.tile([128, 128], BF, tag="mtG", name="mtG")
            nc.vector.memset(mtG[:], 1.0 / H)
            epst = wp.tile([128, 1], F32, tag="epst", name="epst")
            nc.vector.memset(epst[:], EPS)
            wmv = wp.tile([128, NT], BF, tag="wmv", name="wmv")
            nc.vector.memset(wmv[:], 0.0)
            wq = zps.tile([128, NT], F32, tag="zp", name="wq")
            for _ in range(NWARM):
                nc.tensor.matmul(wq[:], mtL[:], wmv[:], start=True, stop=True)

            # ---- input / weight DMAs ----
            # x comes in 2-batch-tile chunks so the per-partition DMA line
            # stays 2KB (bf16 at NT=512 would be 1KB lines = half-rate DMA)
            def x_dma(chunk):
                b0 = chunk * 2 * NT
                xk = []
                for k in range(4):
                    t = xp.tile([128, 2 * NT], BF, tag=f"x{k}", name=f"x{k}")
                    nc.sync.dma_start(out=t[:],
                                      in_=xT[k * 128:(k + 1) * 128,
                                             b0:b0 + 2 * NT])
                    xk.append(t)
                xtl = xp.tile([128, 2 * NT], BF, tag="xt", name="xtl")
                nc.sync.dma_start(out=xtl[0:64, :],
                                  in_=xT[SD:D, b0:b0 + 2 * NT])
                nc.sync.dma_start(out=xtl[64:128, :],
                                  in_=xT[SD:D, b0:b0 + 2 * NT])
                return xk, xtl

            # 3 queues, consumption order.  Critical path for the first
            # m-blocks is x chunk0 (sync) + w1c0 split across gpsimd/scalar;
            # critic 1's w1 lands under critic 0's compute (bt=0 runs the
            # critics sequentially for exactly this reason).
            xchunk = {0: x_dma(0)}
            # per-k w1 DMAs (2KB lines) so the m=0 K-chain starts as soon
            # as its first k-tile lands, alternating gpsimd/scalar
            for k in range(4):
                eng = nc.gpsimd if k % 2 == 0 else nc.scalar
                eng.dma_start(out=w1b[0][:, k, :],
                              in_=w1[0, k * 128:(k + 1) * 128, :])
            nc.gpsimd.dma_start(out=w1tt[:], in_=w1t)
            for k in range(4):
                eng = nc.sync if k % 2 == 0 else nc.scalar
                eng.dma_start(out=w1b[1][:, k, :],
                              in_=w1[1, k * 128:(k + 1) * 128, :])
            for c in range(2):
                eng = nc.gpsimd if c == 0 else nc.scalar
                for j in range(2):
                    eng.dma_start(
                        out=w2b[c][j][:],
                        in_=w2[c, j * 512:(j + 1) * 512, :]
                        .rearrange("(a p) h -> p a h", p=128))
                eng.dma_start(
                    out=wht[c][:],
                    in_=whr[c].rearrange("(a p) h -> p a h", p=128))
                eng.dma_start(out=vt[c][:], in_=vecs[c])
                eng.dma_start(out=qtbv[c * 64:(c + 1) * 64, :], in_=qtb[c])
                eng.dma_start(out=qtbg[0:64, c:c + 1], in_=qtb[c])

            def x_for(bt):
                if bt % 2 == 0 and bt // 2 not in xchunk:
                    xchunk[bt // 2] = x_dma(bt // 2)
                xk, xtl = xchunk[bt // 2]
                off = (bt % 2) * NT
                return [t[:, off:off + NT] for t in xk], xtl, off

            def w1_ap(c, k, m):
                return w1b[c][:, k, m * 128:(m + 1) * 128]

            def w2_ap(c, k, m):
                return w2b[c][k // 4][:, k % 4, m * 128:(m + 1) * 128]

            def b_ap(c, i, m):
                # vt layout [p, vec_idx, m]; vec order: b1,g1,be1,b2,g2,be2
                return vt[c][:, i, m:m + 1]

            def rsqrt(dst, src, bias):
                nc.scalar.activation(dst, src, AF.Abs_reciprocal_sqrt,
                                     bias=bias)

            def relu_evict(c, z, zpm, bias):
                # split across engines: DVE takes critic 0, ACT critic 1,
                # so the two evictions of an m-block run in parallel
                if c == 0:
                    nc.vector.tensor_scalar(z[:], zpm[:], bias, 0.0,
                                            AL.add, AL.max)
                else:
                    nc.scalar.activation(z[:], zpm[:], AF.Relu, bias=bias)

            # rstd1 scales go through a deferred queue drained 2-per-m-block
            # inside the NEXT emission section, so they interleave with that
            # section's PSUM evictions in the DVE FIFO instead of blocking
            # them (head-of-line starvation of the zp ring)
            mul_q = []

            def drain_muls(n):
                for _ in range(min(n, len(mul_q))):
                    z, rs = mul_q.pop(0)
                    nc.vector.tensor_mul(z[:], z[:], rs[:])

            # ---------------- fast path (g == 1, beta == 0) ----------------
            def emit_l1(bt):
                """Layer 1 for both critics; returns the scaled bf16 h tiles
                (mul by rstd1 emitted here, executes under the previous
                tile's L2 stream).  bt=0 runs the critics sequentially with
                the K=64 tails deferred 2 m-blocks (DMA-paced prologue);
                later tiles interleave per m and row-pack the tails."""
                xk, xtlt, off = x_for(bt)
                zs = {0: [None] * NM, 1: [None] * NM}
                sp = {}
                pend = []

                def tail_ap(c, m):
                    return (w1tt[64 * c:64 * c + 64, m * 128:(m + 1) * 128],
                            xtlt[64 * c:64 * c + 64, off:off + NT])

                def evict(c, m, zpm):
                    z = hp.tile([128, NT], BF, tag=f"z{c}{m}",
                                name=f"z{c}{m}", bufs=2)
                    relu_evict(c, z, zpm, b_ap(c, 0, m))
                    if m in STATS_M:
                        z2 = zp_.tile([128, NT], BF, tag=f"z2{c}",
                                      name=f"z2{c}", bufs=2)
                        nc.scalar.activation(z2[:], zpm[:], AF.Square,
                                             bias=b_ap(c, 0, m))
                        pend.append((c, m, z2))
                    zs[c][m] = z

                def flush(upto, c=None):
                    # pair-sum the two sampled z2 m-tiles on gpsimd, then a
                    # single all-(1/256) stats matmul per critic
                    byc = {}
                    for pc, ms, z2 in pend:
                        byc.setdefault(pc, []).append((ms, z2))
                    for pc, entries in byc.items():
                        if c is not None and pc != c:
                            continue
                        if len(entries) < len(STATS_M) or \
                                entries[-1][0] > upto:
                            continue
                        zsum = zp_.tile([128, NT], BF, tag=f"zs{pc}",
                                        name=f"zs{pc}", bufs=2)
                        nc.gpsimd.tensor_add(zsum[:], entries[0][1][:],
                                             entries[1][1][:])
                        sp[pc] = aux.tile([128, NT], F32, tag="aux",
                                          name=f"sp{pc}")
                        nc.tensor.matmul(sp[pc][:], mtL[:], zsum[:],
                                         start=True, stop=True)
                        pend[:] = [e for e in pend if e[0] != pc]

                def scale(c):
                    rs = sp_.tile([128, NT], BF, tag=f"rs{c}", name=f"rs{c}",
                                  bufs=2)
                    rsqrt(rs[:], sp[c][:], epst[:])
                    for m in range(NM):
                        mul_q.append((zs[c][m], rs))

                if bt == 0:
                    for c in range(2):
                        opened = []

                        def finish(m, zpm, c=c):
                            wt, xt = tail_ap(c, m)
                            nc.tensor.matmul(zpm[:], wt, xt, start=False,
                                             stop=True)
                            evict(c, m, zpm)

                        for m in range(NM):
                            drain_muls(2)
                            zpm = zps.tile([128, NT], F32, tag="zp",
                                           name="zp")
                            for k in range(4):
                                nc.tensor.matmul(zpm[:], w1_ap(c, k, m),
                                                 xk[k], start=(k == 0),
                                                 stop=False)
                            opened.append((m, zpm))
                            if len(opened) > 2:
                                finish(*opened.pop(0))
                        while opened:
                            finish(*opened.pop(0))
                        flush(NM, c)
                        scale(c)
                    return zs

                for m in range(NM):
                    drain_muls(2)
                    zpm = {}
                    for c in range(2):
                        zpm[c] = zps.tile([128, NT], F32, tag="zp",
                                          name="zp")
                        for k in range(4):
                            nc.tensor.matmul(zpm[c][:], w1_ap(c, k, m),
                                             xk[k], start=(k == 0),
                                             stop=False)
                    # K=64 action tails, row-packed across the critics
                    for c in range(2):
                        wt, xt = tail_ap(c, m)
                        nc.tensor.matmul(zpm[c][:], wt, xt, start=False,
                                         stop=True)
                    for c in range(2):
                        evict(c, m, zpm[c])
                    flush(m - 2)
                flush(NM)
                for c in range(2):
                    scale(c)
                return zs

            def emit_l2(bt, zs):
                b0 = bt * NT
                S = aux.tile([128, NT], F32, tag="aux", name="S")
                Hb = aux.tile([128, NT], F32, tag="aux", name="Hb")
                ffs = {0: [None] * NM, 1: [None] * NM}
                z2f = {0: [None] * NM, 1: [None] * NM}

                zfp = {0: [None] * (NM // 2), 1: [None] * (NM // 2)}

                def flush(upto):
                    # heads of the two critics are col-disjoint (partitions
                    # 0-63 vs 64-127) and run concurrently; stats likewise,
                    # on pair-summed z2f tiles (4 matmuls per critic)
                    for j in range(flush.done + 1, min(upto, NM - 1) + 1):
                        nc.tensor.matmul(Hb[0:64, :], wht[0][:, j, :],
                                         ffs[0][j][:], start=(j == 0),
                                         stop=(j == NM - 1))
                        nc.tensor.matmul(Hb[64:128, :], wht[1][:, j, :],
                                         ffs[1][j][:], start=(j == 0),
                                         stop=(j == NM - 1))
                        if j % 2 == 1:
                            jj = j // 2
                            nc.tensor.matmul(S[0:64, :], mt64[:],
                                             zfp[0][jj][:], start=(jj == 0),
                                             stop=(jj == NM // 2 - 1))
                            nc.tensor.matmul(S[64:128, :], mt64[:],
                                             zfp[1][jj][:], start=(jj == 0),
                                             stop=(jj == NM // 2 - 1))
                        flush.done = j
                flush.done = -1

                for m in range(NM):
                    drain_muls(2)
                    zpm = {}
                    for c in range(2):
                        zpm[c] = zps.tile([128, NT], F32, tag="zp",
                                          name="zp2")
                        for k in range(NM):
                            nc.tensor.matmul(zpm[c][:], w2_ap(c, k, m),
                                             zs[c][k][:], start=(k == 0),
                                             stop=(k == NM - 1))
                    for c in range(2):
                        ff = hp.tile([128, NT], BF, tag=f"f{c}{m}",
                                     name=f"f{c}{m}", bufs=1)
                        relu_evict(c, ff, zpm[c], b_ap(c, 3, m))
                        zq = zp_.tile([128, NT], BF, tag=f"zf{c}",
                                      name=f"zf{c}", bufs=3)
                        nc.scalar.activation(zq[:], zpm[c][:], AF.Square,
                                             bias=b_ap(c, 3, m))
                        ffs[c][m] = ff
                        z2f[c][m] = zq
                        if m % 2 == 1:
                            zp2t = zp_.tile([128, NT], BF, tag=f"zfp{c}",
                                            name=f"zfp{c}", bufs=2)
                            nc.vector.tensor_add(zp2t[:], z2f[c][m - 1][:],
                                                 zq[:])
                            zfp[c][m // 2] = zp2t
                    flush(m - 2)
                flush(NM)
                # rstd2 lands on the head output (RMS scale invariance);
                # critic c lives on partitions [64c, 64c+64).  Stage-outer
                # + column-halved so the exposed serial tail (last tile)
                # pipelines: rsqrt (ACT) -> mul/bias (DVE) -> DMA per half.
                rs2 = sp_.tile([128, NT], F32, tag="rs2", name="rs2")
                q0 = sp_.tile([128, NT], F32, tag="q0", name="q0")
                qf = sp_.tile([128, NT], F32, tag="qf", name="qf")
                for n0 in (0, NT // 2):
                    cols = slice(n0, n0 + NT // 2)
                    for c in range(2):
                        lo, hi = 64 * c, 64 * c + 64
                        rsqrt(rs2[lo:hi, cols], S[lo:hi, cols],
                              epst[lo:hi, :])
                    for c in range(2):
                        lo, hi = 64 * c, 64 * c + 64
                        nc.vector.tensor_mul(q0[lo:hi, cols], Hb[lo:hi, cols],
                                             rs2[lo:hi, cols])
                        nc.vector.tensor_scalar_add(qf[lo:hi, cols],
                                                    q0[lo:hi, cols],
                                                    qtbv[lo:hi, :])
                        nc.sync.dma_start(
                            out=out_q[c, :, b0 + n0:b0 + n0 + NT // 2],
                            in_=qf[lo:hi, cols])

            def emit_fast():
                carry = None
                for bt in range(NBT + 1):
                    zs = emit_l1(bt) if bt < NBT else None
                    if carry is not None:
                        emit_l2(bt - 1, carry)
                    carry = zs

            # ------------- general path (arbitrary g / beta) -------------
            def gen_block(c, act, wts_of_m, nk, layer, tail=None):
                zs = []
                sp = aux.tile([128, NT], F32, tag="aux", name="sp")
                pend = []

                def flush(upto):
                    while pend and pend[0][0] <= upto:
                        m, z2 = pend.pop(0)
                        nc.tensor.matmul(sp[:], mtG[:], z2[:],
                                         start=(m == 0), stop=(m == NM - 1))

                b_i = 0 if layer == 0 else 3
                for m in range(NM):
                    zpm = zps.tile([128, NT], F32, tag="zp", name="zpg")
                    for k in range(nk):
                        nc.tensor.matmul(zpm[:], wts_of_m(k, m), act[k],
                                         start=(k == 0),
                                         stop=(k == nk - 1 and tail is None))
                    if tail is not None:
                        wt, xt = tail
                        nc.tensor.matmul(zpm[:],
                                         wt[:, m * 128:(m + 1) * 128],
                                         xt, start=False, stop=True)
                    z = zp_.tile([128, NT], F32, tag=f"zg{m}", name=f"zg{m}")
                    nc.vector.tensor_scalar_add(z[:], zpm[:], b_ap(c, b_i, m))
                    z2 = zp_.tile([128, NT], BF, tag=f"z2g_{m % 3}",
                                  name=f"z2g{m % 3}", bufs=1)
                    nc.scalar.activation(z2[:], zpm[:], AF.Square,
                                         bias=b_ap(c, b_i, m))
                    pend.append((m, z2))
                    flush(m - 2)
                    zs.append(z)
                flush(NM)
                return zs, sp

            def gen_norm(c, zs, sp, layer):
                g_i, be_i = (1, 2) if layer == 0 else (4, 5)
                rs = sp_.tile([128, NT], F32, tag="rsg", name="rsg")
                rsqrt(rs[:], sp[:], epst[:])
                hs = []
                for m in range(NM):
                    nc.vector.tensor_mul(zs[m][:], zs[m][:], rs[:])
                    ht = hp.tile([128, NT], BF, tag=f"hg{m}", name=f"hg{m}")
                    nc.scalar.activation(ht[:], zs[m][:], AF.Relu,
                                         bias=b_ap(c, be_i, m),
                                         scale=b_ap(c, g_i, m))
                    hs.append(ht)
                return hs

            def emit_general():
                for bt in range(NBT):
                    b0 = bt * NT
                    xk, xtlt, off = x_for(bt)
                    for c in range(2):
                        tail = (w1tt[64 * c:64 * c + 64, :],
                                xtlt[64 * c:64 * c + 64, off:off + NT])
                        zs, sp = gen_block(
                            c, xk, lambda k, m, c=c: w1_ap(c, k, m), 4, 0,
                            tail=tail)
                        h1 = gen_norm(c, zs, sp, 0)
                        zs, sp = gen_block(
                            c, [t[:] for t in h1],
                            lambda k, m, c=c: w2_ap(c, k, m), NM, 1)
                        ff = gen_norm(c, zs, sp, 1)
                        qp = aux.tile([128, NT], F32, tag="aux", name="qp")
                        for k in range(NM):
                            nc.tensor.matmul(qp[0:64, :], wht[c][:, k, :],
                                             ff[k][:], start=(k == 0),
                                             stop=(k == NM - 1))
                        qf = sp_.tile([128, NT], F32, tag="qfg", name="qfg",
                                      bufs=2)
                        nc.scalar.activation(qf[0:64, :], qp[0:64, :],
                                             AF.Identity,
                                             bias=qtbg[0:64, c:c + 1])
                        nc.gpsimd.dma_start(out=out_q[c, :, b0:b0 + NT],
                                            in_=qf[0:64, :])

            if unit_affine:
                emit_fast()
            else:
                emit_general()
    nc.compile()
    return nc


def _prep_host(inputs):
    state = np.ascontiguousarray(inputs["state"], dtype=np.float32)
    action = np.ascontiguousarray(inputs["action"], dtype=np.float32)
    W_f1 = np.asarray(inputs["W_f1"], np.float32)
    b_f1 = np.asarray(inputs["b_f1"], np.float32)
    g1 = np.asarray(inputs["g1"], np.float32)
    beta1 = np.asarray(inputs["beta1"], np.float32)
    W_f2 = np.asarray(inputs["W_f2"], np.float32)
    b_f2 = np.asarray(inputs["b_f2"], np.float32)
    g2 = np.asarray(inputs["g2"], np.float32)
    beta2 = np.asarray(inputs["beta2"], np.float32)
    W_h = np.asarray(inputs["W_h"], np.float32)
    b_h = np.asarray(inputs["b_h"], np.float32)
    W_e1 = np.asarray(inputs["W_e1"], np.float32)
    b_e1 = np.asarray(inputs["b_e1"], np.float32)
    W_e2 = np.asarray(inputs["W_e2"], np.float32)
    b_e2 = np.asarray(inputs["b_e2"], np.float32)

    unit_affine = (np.all(g1 == 1.0) and np.all(beta1 == 0.0)
                   and np.all(g2 == 1.0) and np.all(beta2 == 0.0))

    x = np.concatenate([state, action], axis=1)          # [B, 576]
    xT = np.ascontiguousarray(x.T).astype(BF16_NP)       # [576, B] bf16

    # transpose weights and fold the LN mean subtraction into them:
    # centering the columns of W.T (and the bias) makes mean_h(z) == 0.
    w1tr = np.ascontiguousarray(W_f1.transpose(0, 2, 1))  # [2, D, H]
    w1c = w1tr - w1tr.mean(axis=2, keepdims=True)
    b1c = b_f1 - b_f1.mean(axis=1, keepdims=True)         # [2, H]
    w2tr = np.ascontiguousarray(W_f2.transpose(0, 2, 1))  # [2, H, H]
    w2c = w2tr - w2tr.mean(axis=2, keepdims=True)
    b2c = b_f2 - b_f2.mean(axis=1, keepdims=True)         # [2, H]

    w1main = np.ascontiguousarray(w1c[:, :SD, :]).astype(BF16_NP)
    w1tail = np.ascontiguousarray(
        np.concatenate([w1c[0, SD:D, :], w1c[1, SD:D, :]], axis=0)
    ).astype(BF16_NP)                                     # [128, H]
    w2b = np.ascontiguousarray(w2c).astype(BF16_NP)

    def as_pm(v):                                        # [2, H] -> [2,128,NM]
        return v.reshape(2, NM, 128).transpose(0, 2, 1)

    vecs = np.ascontiguousarray(np.stack(
        [as_pm(b1c), as_pm(g1), as_pm(beta1),
         as_pm(b2c), as_pm(g2), as_pm(beta2)],
        axis=1).transpose(0, 2, 1, 3))                   # [2, 128, 6, NM]

    wh_feat = W_h[:, 0, :H]                              # [2, H]
    whr = np.ascontiguousarray(
        np.broadcast_to(wh_feat[:, :, None], (2, H, 64)).copy()
    ).astype(BF16_NP)

    # tau embedding: batch-independent, tiny -> host
    tau = (np.linspace(0.0, 1.0, NQ + 1, dtype=np.float32)[:-1]
           + np.float32(1.0 / (2 * NQ)))[:, None]        # [NQ, 1]
    qtb = np.empty((2, 64, 1), np.float32)
    for c in range(2):
        te = np.maximum(tau @ W_e1[c].T + b_e1[c], 0.0) @ W_e2[c].T + b_e2[c]
        qtb[c, :, 0] = te @ W_h[c, 0, H:] + b_h[c, 0]

    shared = {"w1": w1main, "w1t": w1tail, "w2": w2b, "whr": whr,
              "vecs": np.ascontiguousarray(vecs), "qtb": qtb}
    return xT, shared, unit_affine


def kernel(**inputs):
    global _LAST_RESULT
    xT, shared, unit_affine = _prep_host(inputs)
    key = ("nc", unit_affine)
    if key not in _CACHE:
        _CACHE[key] = _build(unit_affine)
    nc = _CACHE[key]

    in_maps = []
    for c in range(NCORES):
        m = dict(shared)
        m["xT"] = np.ascontiguousarray(xT[:, c * BSH:(c + 1) * BSH])
        in_maps.append(m)

    trace = bool(os.environ.get("KERNEL_TRACE"))
    res = run_bass_kernel_spmd(nc, in_maps, list(range(NCORES)), trace=trace)
    _LAST_RESULT = res

    q = np.concatenate([res.results[i]["out_q"] for i in range(NCORES)],
                       axis=2)                           # [2, NQ, B]
    q = np.ascontiguousarray(q.transpose(0, 2, 1))       # [2, B, NQ]
    return q[0], q[1]
